# revision 15
# baseline (speedup 1.0000x reference)
"""Trainium2 Bass kernel for a 3-layer GNN message-passing block.

Reference computation (per layer i):
    x1 = h @ Wfc[i] + bfc[i]                        # [N_SUB, D]
    x2 = scatter_mean(h, idx) @ Wsum[i] + bsum[i]   # [NUM_GRAPHS, D]
    h  = elu(x1 + x2[idx])
then
    out = relu(scatter_mean(h, idx) @ Wf1 + bf1) @ Wf2 + bf2

Strategy: data-parallel over 8 NeuronCores. The sorted graph index lets us
split graphs contiguously; each core owns 5 "windows" of <=128 graphs /
<=2560 subgraph rows (rows padded + permuted host-side so every window is
exactly 20 chunks of 128 rows). scatter_mean and the x2[idx] gather are both
expressed as one-hot matmuls on the TensorEngine, with the one-hot matrices
built host-side from the index. All shapes/structure are compile-time
constants; the same program runs SPMD on all 8 cores with per-core data.
"""

import numpy as np

P = 128
D = 512
N_SUB = 100000
NUM_GRAPHS = 4096
N_LAYERS = 3
NUM_TASKS = 10
N_CORES = 8
WIN_PER_CORE = 5
CH_PER_WIN = 20
ROWS_PER_WIN = CH_PER_WIN * P            # 2560
N_LOC = WIN_PER_CORE * ROWS_PER_WIN      # 12800 padded rows per core
CHUNKS = N_LOC // P                      # 100
G_WIN = P                                # graph slots per window
G_LOC = WIN_PER_CORE * G_WIN             # 640 graph slots per core
N_WIN_TOTAL = N_CORES * WIN_PER_CORE     # 40
DBLK = D // P                            # 4
D2 = 2 * D                               # 1024
D2BLK = D2 // P                          # 8

_cached = {}


# ----------------------------------------------------------------- host prep

def _pack_windows(counts):
    """Split graphs 0..NUM_GRAPHS-1 into N_WIN_TOTAL contiguous windows with
    <= G_WIN graphs and <= ROWS_PER_WIN rows each, roughly row-balanced."""
    total = int(counts.sum())
    target = total / N_WIN_TOTAL
    wins = []
    g = 0
    rows_done = 0
    for w in range(N_WIN_TOTAL):
        g0 = g
        rows_w = 0
        while g < NUM_GRAPHS:
            c = int(counts[g])
            if rows_w + c > ROWS_PER_WIN or (g - g0) >= G_WIN:
                break
            if (w < N_WIN_TOTAL - 1 and rows_w > 0
                    and rows_done + rows_w + c > (w + 1) * target):
                remaining = total - (rows_done + rows_w)
                if remaining <= (N_WIN_TOTAL - w - 1) * ROWS_PER_WIN * 0.98:
                    break
            rows_w += c
            g += 1
        while g < NUM_GRAPHS and counts[g] == 0 and (g - g0) < G_WIN:
            g += 1
        rows_done += rows_w
        wins.append((g0, g))
    assert g == NUM_GRAPHS, f"window packing failed: {g}/{NUM_GRAPHS}"
    return wins


def _build_core_inputs(h, idx, counts, starts, wins, core, shared):
    h_pad = np.zeros((N_LOC, D), dtype=np.float16)
    slot = np.full(N_LOC, -1, dtype=np.int64)
    invc = np.zeros((P, WIN_PER_CORE), dtype=np.float32)  # [g_in_win, w]
    gmap = []
    for lw in range(WIN_PER_CORE):
        g0, g1 = wins[core * WIN_PER_CORE + lw]
        r0, r1 = int(starts[g0]), int(starts[g1])
        n = r1 - r0
        h_pad[lw * ROWS_PER_WIN: lw * ROWS_PER_WIN + n] = h[r0:r1]
        slot[lw * ROWS_PER_WIN: lw * ROWS_PER_WIN + n] = \
            lw * G_WIN + (idx[r0:r1] - g0)
        for j, g in enumerate(range(g0, g1)):
            invc[j, lw] = 1.0 / max(int(counts[g]), 1)
            gmap.append((g, lw * G_WIN + j))
    wsc = np.zeros((CHUNKS, P, P), dtype=np.float16)
    for c in range(CHUNKS):
        w = c // CH_PER_WIN
        s = slot[c * P:(c + 1) * P]
        real = np.nonzero(s >= 0)[0]
        wsc[c][real, s[real] - w * G_WIN] = 1.0
    wga = np.transpose(wsc, (0, 2, 1))
    # sbuf layout [P, CHUNKS*P]: partition p holds chunk-c block at cols 128c..
    wsc_flat = np.ascontiguousarray(np.transpose(wsc, (1, 0, 2)).reshape(P, CHUNKS * P))
    wga_flat = np.ascontiguousarray(np.transpose(wga, (1, 0, 2)).reshape(P, CHUNKS * P))
    in_map = {
        "h16": h_pad,
        "wsc": wsc_flat,
        "wga": wga_flat,
        "invc": invc,
        **shared,
    }
    return in_map, gmap


def _prep_shared(Wfc, bfc, Wsum, bsum, Wf1, bf1, Wf2, bf2):
    f16 = np.float16
    wfc = np.stack([
        np.stack([Wfc[i, b * P:(b + 1) * P, :] for b in range(DBLK)])
        for i in range(N_LAYERS)
    ]).astype(f16)                                             # [3,4,128,512]
    wsum = np.stack([
        np.stack([Wsum[i, b * P:(b + 1) * P, :] for b in range(DBLK)])
        for i in range(N_LAYERS)
    ]).astype(f16)
    cbias = (bfc + bsum).astype(f16)[:, None, :]               # [3,1,512]
    wf1 = np.stack([
        np.stack([Wf1[b * P:(b + 1) * P, q * P:(q + 1) * P] for q in range(D2BLK)])
        for b in range(DBLK)
    ]).astype(f16)                                             # [4,8,128,128]
    bf1w = bf1.astype(f16)[None, :]                            # [1,1024]
    wf2 = np.stack([Wf2[q * P:(q + 1) * P, :] for q in range(D2BLK)]).astype(f16)
    bf2w = bf2.astype(f16)[None, :]                            # [1,10]
    return {
        "wfc": wfc, "wsum": wsum, "cbias": cbias,
        "wf1": wf1, "bf1w": bf1w, "wf2": wf2, "bf2w": bf2w,
    }


# -------------------------------------------------------------- bass program

def _build_program():
    from contextlib import ExitStack

    import concourse.mybir as mybir
    import concourse.tile as tile
    from concourse import bacc
    from concourse.masks import make_identity

    f16 = mybir.dt.float16
    f32 = mybir.dt.float32
    AF = mybir.ActivationFunctionType
    ALU = mybir.AluOpType

    nc = bacc.Bacc("TRN2", debug=False, target_bir_lowering=False,
                   num_devices=N_CORES, dynamic_dma_scratch_size=2048)

    # DRAM tensors
    h16_d = nc.dram_tensor("h16", [N_LOC, D], f16, kind="ExternalInput")
    wsc_d = nc.dram_tensor("wsc", [P, CHUNKS * P], f16, kind="ExternalInput")
    wga_d = nc.dram_tensor("wga", [P, CHUNKS * P], f16, kind="ExternalInput")
    invc_d = nc.dram_tensor("invc", [P, WIN_PER_CORE], f32, kind="ExternalInput")
    wfc_d = nc.dram_tensor("wfc", [N_LAYERS, DBLK, P, D], f16, kind="ExternalInput")
    wsum_d = nc.dram_tensor("wsum", [N_LAYERS, DBLK, P, D], f16, kind="ExternalInput")
    cbias_d = nc.dram_tensor("cbias", [N_LAYERS, 1, D], f16, kind="ExternalInput")
    wf1_d = nc.dram_tensor("wf1", [DBLK, D2BLK, P, P], f16, kind="ExternalInput")
    bf1_d = nc.dram_tensor("bf1w", [1, D2], f16, kind="ExternalInput")
    wf2_d = nc.dram_tensor("wf2", [D2BLK, P, NUM_TASKS], f16, kind="ExternalInput")
    bf2_d = nc.dram_tensor("bf2w", [1, NUM_TASKS], f16, kind="ExternalInput")
    out_d = nc.dram_tensor("out", [NUM_TASKS, G_LOC], f32, kind="ExternalOutput")

    with tile.TileContext(nc) as tc, ExitStack() as ctx:
        const = ctx.enter_context(tc.tile_pool(name="const", bufs=1))
        hpool = ctx.enter_context(tc.tile_pool(name="h", bufs=1))
        stream = ctx.enter_context(tc.tile_pool(name="stream", bufs=8))
        work = ctx.enter_context(tc.tile_pool(name="work", bufs=2))
        x2pool = ctx.enter_context(tc.tile_pool(name="x2", bufs=2))
        psum = ctx.enter_context(tc.tile_pool(name="psum", bufs=2, space="PSUM"))
        psx1 = ctx.enter_context(tc.tile_pool(name="psx1", bufs=2, space="PSUM"))

        # ---- constants (each written exactly once; no DMA slot reuse)
        ident = const.tile([P, P], f16, tag="ident")
        make_identity(nc, ident[:])
        ones = const.tile([1, P], f16, tag="ones")
        nc.vector.memset(ones[:], 1.0)
        invc_t = const.tile([P, WIN_PER_CORE], f32, tag="invc")
        nc.sync.dma_start(invc_t[:], invc_d[:, :])
        wscb = const.tile([P, CHUNKS * P], f16, tag="wscb")
        nc.sync.dma_start(wscb[:], wsc_d[:, :])
        wgab = const.tile([P, CHUNKS * P], f16, tag="wgab")
        nc.sync.dma_start(wgab[:], wga_d[:, :])
        wfc_t = [[const.tile([P, D], f16, tag=f"wfc{i}_{b}", name=f"wfc{i}_{b}")
                  for b in range(DBLK)] for i in range(N_LAYERS)]
        wsum_t = [[const.tile([P, D], f16, tag=f"wsum{i}_{b}", name=f"wsum{i}_{b}")
                   for b in range(DBLK)] for i in range(N_LAYERS)]
        cbias_t = [const.tile([1, D], f16, tag=f"cb{i}", name=f"cb{i}")
                   for i in range(N_LAYERS)]
        for i in range(N_LAYERS):
            for b in range(DBLK):
                nc.sync.dma_start(wfc_t[i][b][:], wfc_d[i, b])
                nc.sync.dma_start(wsum_t[i][b][:], wsum_d[i, b])
            nc.sync.dma_start(cbias_t[i][:], cbias_d[i])
        wf1_t = [[const.tile([P, P], f16, tag=f"wf1_{b}_{q}", name=f"wf1_{b}_{q}")
                  for q in range(D2BLK)] for b in range(DBLK)]
        for b in range(DBLK):
            for q in range(D2BLK):
                nc.sync.dma_start(wf1_t[b][q][:], wf1_d[b, q])
        bf1_t = const.tile([1, D2], f16, tag="bf1")
        nc.sync.dma_start(bf1_t[:], bf1_d[:, :])
        wf2_t = [const.tile([P, NUM_TASKS], f16, tag=f"wf2_{q}", name=f"wf2_{q}")
                 for q in range(D2BLK)]
        for q in range(D2BLK):
            nc.sync.dma_start(wf2_t[q][:], wf2_d[q])
        bf2_t = const.tile([1, NUM_TASKS], f16, tag="bf2")
        nc.sync.dma_start(bf2_t[:], bf2_d[:, :])

        # ---- resident h tiles (each written once by DMA, then by DVE)
        h_t = []
        for c in range(CHUNKS):
            t = hpool.tile([P, D], f16, tag=f"h{c}", name=f"h{c}")
            nc.sync.dma_start(t[:], h16_d[c * P:(c + 1) * P, :])
            h_t.append(t)

        def wsc_c(c):
            return wscb[:, c * P:(c + 1) * P]

        def wga_c(c):
            return wgab[:, c * P:(c + 1) * P]

        def scatter_window(w):
            """Segment-sum window w's 20 chunks -> mean [g, d] -> meanT f16."""
            ps = psum.tile([P, D], f32, tag="sc")
            for i in range(CH_PER_WIN):
                c = w * CH_PER_WIN + i
                nc.tensor.matmul(ps[:], lhsT=wsc_c(c), rhs=h_t[c][:],
                                 start=(i == 0), stop=(i == CH_PER_WIN - 1))
            mean = work.tile([P, D], f16, tag="mean")
            nc.scalar.activation(mean[:], ps[:], AF.Copy,
                                 scale=invc_t[:, w:w + 1])
            pst = psum.tile([P, D], f16, tag="tr")
            for b in range(DBLK):
                nc.tensor.transpose(pst[:, b * P:(b + 1) * P],
                                    mean[:, b * P:(b + 1) * P], ident[:])
            meanT = work.tile([P, D], f16, tag="meanT")
            nc.scalar.activation(meanT[:], pst[:], AF.Copy)
            return meanT

        def x2_window(meanT, w, layer):
            """x2 = meanT.T @ Wsum + (bfc+bsum), as f16 [g, d]."""
            ps = psum.tile([P, D], f32, tag="x2")
            for b in range(DBLK):
                nc.tensor.matmul(ps[:], lhsT=meanT[:, b * P:(b + 1) * P],
                                 rhs=wsum_t[layer][b][:],
                                 start=(b == 0), stop=False)
            nc.tensor.matmul(ps[:], lhsT=ones[:, :P], rhs=cbias_t[layer][:],
                             start=False, stop=True)
            x2w = x2pool.tile([P, D], f16, tag=f"x2w{w}", name=f"x2w{w}", bufs=1)
            nc.scalar.activation(x2w[:], ps[:], AF.Copy)
            return x2w

        for layer in range(N_LAYERS):
            # scatter + x2 per window (pipelined one window deep)
            meanTs = {}
            x2ws = {}
            for w in range(WIN_PER_CORE):
                meanTs[w] = scatter_window(w)
                if w >= 1:
                    x2ws[w - 1] = x2_window(meanTs.pop(w - 1), w - 1, layer)
            x2ws[WIN_PER_CORE - 1] = x2_window(
                meanTs.pop(WIN_PER_CORE - 1), WIN_PER_CORE - 1, layer)

            # update pass
            for c in range(CHUNKS):
                w = c // CH_PER_WIN
                hts = []
                for b in range(DBLK):
                    ht = stream.tile([P, P], f16, tag="hT", name=f"hT{c}_{b}")
                    nc.sync.dma_start(ht[:], h_t[c][:, b * P:(b + 1) * P],
                                      transpose=True)
                    hts.append(ht)
                ps = psx1.tile([P, D], f32, tag="x1")
                for b in range(DBLK):
                    nc.tensor.matmul(ps[:], lhsT=hts[b][:],
                                     rhs=wfc_t[layer][b][:],
                                     start=(b == 0), stop=False)
                nc.tensor.matmul(ps[:], lhsT=wga_c(c), rhs=x2ws[w][:],
                                 start=False, stop=True)
                # ELU: h = relu(z) - 1 + exp(min(z, 0))
                t1 = work.tile([P, D], f16, tag="t1")
                nc.scalar.activation(t1[:], ps[:], AF.Relu, scale=-1.0)
                t2 = work.tile([P, D], f16, tag="t2")
                nc.scalar.activation(t2[:], t1[:], AF.Exp, scale=-1.0)
                pm1 = work.tile([P, D], f16, tag="pm1")
                nc.vector.tensor_scalar(pm1[:], ps[:], 0.0, -1.0,
                                        op0=ALU.max, op1=ALU.add)
                nc.vector.tensor_tensor(h_t[c][:], pm1[:], t2[:], op=ALU.add)

        # ---- final scatter + head
        out_sb = const.tile([NUM_TASKS, G_LOC], f32, tag="out")
        for w in range(WIN_PER_CORE):
            hgT = scatter_window(w)            # [d, g] blocks, f16
            # t.T = relu(Wf1.T @ hg.T + bf1)  -> [d2, g] blocks
            tT = work.tile([P, D2], f16, tag="tT", bufs=1)
            for half in range(2):
                ps = psx1.tile([P, D], f32, tag="x1")
                for qi in range(4):
                    q = half * 4 + qi
                    sl = slice(qi * P, (qi + 1) * P)
                    for b in range(DBLK):
                        nc.tensor.matmul(ps[:, sl],
                                         lhsT=wf1_t[b][q][:],
                                         rhs=hgT[:, b * P:(b + 1) * P],
                                         start=(b == 0), stop=False)
                    nc.tensor.matmul(ps[:, sl],
                                     lhsT=bf1_t[:, q * P:(q + 1) * P],
                                     rhs=ones[:, :P], start=False, stop=True)
                nc.scalar.activation(tT[:, half * D:(half + 1) * D],
                                     ps[:], AF.Relu)
            pso = psum.tile([NUM_TASKS, P], f32, tag="x2")
            for q in range(D2BLK):
                nc.tensor.matmul(pso[:], lhsT=wf2_t[q][:],
                                 rhs=tT[:, q * P:(q + 1) * P],
                                 start=(q == 0), stop=False)
            nc.tensor.matmul(pso[:], lhsT=bf2_t[:], rhs=ones[:, :P],
                             start=False, stop=True)
            nc.vector.tensor_copy(out_sb[:, w * P:(w + 1) * P], pso[:])
        nc.sync.dma_start(out_d[:, :], out_sb[:])

    nc.compile()
    return nc


# ------------------------------------------------------------------- kernel

def kernel(**inputs):
    h = np.asarray(inputs["h_subgraph"], dtype=np.float32)
    idx = np.asarray(inputs["subgraph_idx_batch"]).astype(np.int64)
    if not np.all(idx[:-1] <= idx[1:]):        # defensive: index must be sorted
        order = np.argsort(idx, kind="stable")
        h, idx = h[order], idx[order]

    counts = np.bincount(idx, minlength=NUM_GRAPHS)
    starts = np.concatenate([[0], np.cumsum(counts)])
    wins = _pack_windows(counts)
    shared = _prep_shared(
        np.asarray(inputs["Wfc"], np.float32), np.asarray(inputs["bfc"], np.float32),
        np.asarray(inputs["Wsum"], np.float32), np.asarray(inputs["bsum"], np.float32),
        np.asarray(inputs["Wf1"], np.float32), np.asarray(inputs["bf1"], np.float32),
        np.asarray(inputs["Wf2"], np.float32), np.asarray(inputs["bf2"], np.float32),
    )

    in_maps = []
    gmaps = []
    for core in range(N_CORES):
        m, gm = _build_core_inputs(h, idx, counts, starts, wins, core, shared)
        in_maps.append(m)
        gmaps.append(gm)

    _cached["in_maps"] = in_maps
    if "nc" not in _cached:
        _cached["nc"] = _build_program()
    nc = _cached["nc"]

    from concourse import bass_utils
    res = bass_utils.run_bass_kernel_spmd(
        nc, in_maps, core_ids=list(range(N_CORES)))

    out = np.zeros((NUM_GRAPHS, NUM_TASKS), dtype=np.float32)
    for core in range(N_CORES):
        o = res.results[core]["out"]           # [10, 640]
        for g, s in gmaps[core]:
            out[g] = o[:, s]
    return out


# revision 19
# speedup vs baseline: 2.2923x; 2.2923x over previous
"""Trainium2 Bass kernel for a 3-layer GNN message-passing block.

Reference computation (per layer i):
    x1 = h @ Wfc[i] + bfc[i]                        # [N_SUB, D]
    x2 = scatter_mean(h, idx) @ Wsum[i] + bsum[i]   # [NUM_GRAPHS, D]
    h  = elu(x1 + x2[idx])
then
    out = relu(scatter_mean(h, idx) @ Wf1 + bf1) @ Wf2 + bf2

Strategy: data-parallel over 8 NeuronCores. The sorted graph index lets us
split graphs contiguously; each core owns 5 "windows" of <=128 graphs /
<=2560 subgraph rows (rows padded + permuted host-side so every window is
exactly 20 chunks of 128 rows). scatter_mean and the x2[idx] gather are both
expressed as one-hot matmuls on the TensorEngine, with the one-hot matrices
built host-side from the index. All shapes/structure are compile-time
constants; the same program runs SPMD on all 8 cores with per-core data.
"""

import numpy as np

P = 128
D = 512
N_SUB = 100000
NUM_GRAPHS = 4096
N_LAYERS = 3
NUM_TASKS = 10
N_CORES = 8
WIN_PER_CORE = 5
CH_PER_WIN = 20
ROWS_PER_WIN = CH_PER_WIN * P            # 2560
N_LOC = WIN_PER_CORE * ROWS_PER_WIN      # 12800 padded rows per core
CHUNKS = N_LOC // P                      # 100
G_WIN = P                                # graph slots per window
G_LOC = WIN_PER_CORE * G_WIN             # 640 graph slots per core
N_WIN_TOTAL = N_CORES * WIN_PER_CORE     # 40
DBLK = D // P                            # 4
D2 = 2 * D                               # 1024
D2BLK = D2 // P                          # 8

_cached = {}


# ----------------------------------------------------------------- host prep

def _pack_windows(counts):
    """Split graphs 0..NUM_GRAPHS-1 into N_WIN_TOTAL contiguous windows with
    <= G_WIN graphs and <= ROWS_PER_WIN rows each, roughly row-balanced."""
    total = int(counts.sum())
    target = total / N_WIN_TOTAL
    wins = []
    g = 0
    rows_done = 0
    for w in range(N_WIN_TOTAL):
        g0 = g
        rows_w = 0
        while g < NUM_GRAPHS:
            c = int(counts[g])
            if rows_w + c > ROWS_PER_WIN or (g - g0) >= G_WIN:
                break
            if (w < N_WIN_TOTAL - 1 and rows_w > 0
                    and rows_done + rows_w + c > (w + 1) * target):
                remaining = total - (rows_done + rows_w)
                if remaining <= (N_WIN_TOTAL - w - 1) * ROWS_PER_WIN * 0.98:
                    break
            rows_w += c
            g += 1
        while g < NUM_GRAPHS and counts[g] == 0 and (g - g0) < G_WIN:
            g += 1
        rows_done += rows_w
        wins.append((g0, g))
    assert g == NUM_GRAPHS, f"window packing failed: {g}/{NUM_GRAPHS}"
    return wins


def _build_core_inputs(h, idx, counts, starts, wins, core, shared):
    h_pad = np.zeros((N_LOC, D), dtype=np.float16)
    slot = np.full(N_LOC, -1, dtype=np.int64)
    invc = np.zeros((P, WIN_PER_CORE), dtype=np.float32)  # [g_in_win, w]
    gmap = []
    for lw in range(WIN_PER_CORE):
        g0, g1 = wins[core * WIN_PER_CORE + lw]
        r0, r1 = int(starts[g0]), int(starts[g1])
        n = r1 - r0
        h_pad[lw * ROWS_PER_WIN: lw * ROWS_PER_WIN + n] = h[r0:r1]
        slot[lw * ROWS_PER_WIN: lw * ROWS_PER_WIN + n] = \
            lw * G_WIN + (idx[r0:r1] - g0)
        for j, g in enumerate(range(g0, g1)):
            invc[j, lw] = 1.0 / max(int(counts[g]), 1)
            gmap.append((g, lw * G_WIN + j))
    wsc = np.zeros((CHUNKS, P, P), dtype=np.float16)
    for c in range(CHUNKS):
        w = c // CH_PER_WIN
        s = slot[c * P:(c + 1) * P]
        real = np.nonzero(s >= 0)[0]
        wsc[c][real, s[real] - w * G_WIN] = 1.0
    wga = np.transpose(wsc, (0, 2, 1))
    # sbuf layout [P, CHUNKS*P]: partition p holds chunk-c block at cols 128c..
    wsc_flat = np.ascontiguousarray(np.transpose(wsc, (1, 0, 2)).reshape(P, CHUNKS * P))
    wga_flat = np.ascontiguousarray(np.transpose(wga, (1, 0, 2)).reshape(P, CHUNKS * P))
    in_map = {
        "h16": h_pad,
        "wsc": wsc_flat,
        "wga": wga_flat,
        "invc": invc,
        **shared,
    }
    return in_map, gmap


def _prep_shared(Wfc, bfc, Wsum, bsum, Wf1, bf1, Wf2, bf2):
    f16 = np.float16
    wfc = np.stack([
        np.stack([Wfc[i, b * P:(b + 1) * P, :] for b in range(DBLK)])
        for i in range(N_LAYERS)
    ]).astype(f16)                                             # [3,4,128,512]
    wsum = np.stack([
        np.stack([Wsum[i, b * P:(b + 1) * P, :] for b in range(DBLK)])
        for i in range(N_LAYERS)
    ]).astype(f16)
    cbias = (bfc + bsum).astype(f16)[:, None, :]               # [3,1,512]
    wf1 = np.stack([
        np.stack([Wf1[b * P:(b + 1) * P, q * P:(q + 1) * P] for q in range(D2BLK)])
        for b in range(DBLK)
    ]).astype(f16)                                             # [4,8,128,128]
    bf1w = bf1.astype(f16)[None, :]                            # [1,1024]
    wf2 = np.stack([Wf2[q * P:(q + 1) * P, :] for q in range(D2BLK)]).astype(f16)
    bf2w = bf2.astype(f16)[None, :]                            # [1,10]
    return {
        "wfc": wfc, "wsum": wsum, "cbias": cbias,
        "wf1": wf1, "bf1w": bf1w, "wf2": wf2, "bf2w": bf2w,
    }


# -------------------------------------------------------------- bass program

def _build_program():
    from contextlib import ExitStack

    import concourse.mybir as mybir
    import concourse.tile as tile
    from concourse import bacc
    from concourse.masks import make_identity

    f16 = mybir.dt.float16
    f32 = mybir.dt.float32
    AF = mybir.ActivationFunctionType
    ALU = mybir.AluOpType

    nc = bacc.Bacc("TRN2", debug=False, target_bir_lowering=False,
                   num_devices=N_CORES, dynamic_dma_scratch_size=2048)

    # DRAM tensors
    h16_d = nc.dram_tensor("h16", [N_LOC, D], f16, kind="ExternalInput")
    wsc_d = nc.dram_tensor("wsc", [P, CHUNKS * P], f16, kind="ExternalInput")
    wga_d = nc.dram_tensor("wga", [P, CHUNKS * P], f16, kind="ExternalInput")
    invc_d = nc.dram_tensor("invc", [P, WIN_PER_CORE], f32, kind="ExternalInput")
    wfc_d = nc.dram_tensor("wfc", [N_LAYERS, DBLK, P, D], f16, kind="ExternalInput")
    wsum_d = nc.dram_tensor("wsum", [N_LAYERS, DBLK, P, D], f16, kind="ExternalInput")
    cbias_d = nc.dram_tensor("cbias", [N_LAYERS, 1, D], f16, kind="ExternalInput")
    wf1_d = nc.dram_tensor("wf1", [DBLK, D2BLK, P, P], f16, kind="ExternalInput")
    bf1_d = nc.dram_tensor("bf1w", [1, D2], f16, kind="ExternalInput")
    wf2_d = nc.dram_tensor("wf2", [D2BLK, P, NUM_TASKS], f16, kind="ExternalInput")
    bf2_d = nc.dram_tensor("bf2w", [1, NUM_TASKS], f16, kind="ExternalInput")
    out_d = nc.dram_tensor("out", [NUM_TASKS, G_LOC], f32, kind="ExternalOutput")

    with tile.TileContext(nc) as tc, ExitStack() as ctx:
        const = ctx.enter_context(tc.tile_pool(name="const", bufs=1))
        hpool = ctx.enter_context(tc.tile_pool(name="h", bufs=1))
        stream = ctx.enter_context(tc.tile_pool(name="stream", bufs=6))
        work = ctx.enter_context(tc.tile_pool(name="work", bufs=2))
        x2pool = ctx.enter_context(tc.tile_pool(name="x2", bufs=2))
        psum = ctx.enter_context(tc.tile_pool(name="psum", bufs=2, space="PSUM"))
        psx1 = ctx.enter_context(tc.tile_pool(name="psx1", bufs=2, space="PSUM"))

        # ---- constants (each written exactly once; no DMA slot reuse)
        ident = const.tile([P, P], f16, tag="ident")
        make_identity(nc, ident[:])
        ones = const.tile([1, P], f16, tag="ones")
        nc.vector.memset(ones[:], 1.0)
        invc_t = const.tile([P, WIN_PER_CORE], f32, tag="invc")
        nc.sync.dma_start(invc_t[:], invc_d[:, :])
        wscb = const.tile([P, CHUNKS * P], f16, tag="wscb")
        nc.sync.dma_start(wscb[:], wsc_d[:, :])
        wgab = const.tile([P, CHUNKS * P], f16, tag="wgab")
        nc.sync.dma_start(wgab[:], wga_d[:, :])
        wfc_t = [[const.tile([P, D], f16, tag=f"wfc{i}_{b}", name=f"wfc{i}_{b}")
                  for b in range(DBLK)] for i in range(N_LAYERS)]
        wsum_t = [[const.tile([P, D], f16, tag=f"wsum{i}_{b}", name=f"wsum{i}_{b}")
                   for b in range(DBLK)] for i in range(N_LAYERS)]
        cbias_t = [const.tile([1, D], f16, tag=f"cb{i}", name=f"cb{i}")
                   for i in range(N_LAYERS)]
        for i in range(N_LAYERS):
            for b in range(DBLK):
                nc.sync.dma_start(wfc_t[i][b][:], wfc_d[i, b])
                nc.sync.dma_start(wsum_t[i][b][:], wsum_d[i, b])
            nc.sync.dma_start(cbias_t[i][:], cbias_d[i])
        wf1_t = [[const.tile([P, P], f16, tag=f"wf1_{b}_{q}", name=f"wf1_{b}_{q}")
                  for q in range(D2BLK)] for b in range(DBLK)]
        for b in range(DBLK):
            for q in range(D2BLK):
                nc.sync.dma_start(wf1_t[b][q][:], wf1_d[b, q])
        bf1_t = const.tile([1, D2], f16, tag="bf1")
        nc.sync.dma_start(bf1_t[:], bf1_d[:, :])
        wf2_t = [const.tile([P, NUM_TASKS], f16, tag=f"wf2_{q}", name=f"wf2_{q}")
                 for q in range(D2BLK)]
        for q in range(D2BLK):
            nc.sync.dma_start(wf2_t[q][:], wf2_d[q])
        bf2_t = const.tile([1, NUM_TASKS], f16, tag="bf2")
        nc.sync.dma_start(bf2_t[:], bf2_d[:, :])

        # ---- resident h tiles (each written once by DMA, then by DVE).
        # SWDGE (gpsimd) keeps these 100 issues off the SP sequencer, which
        # handles the other constant loads.
        h_t = []
        for c in range(CHUNKS):
            t = hpool.tile([P, D], f16, tag=f"h{c}", name=f"h{c}")
            nc.sync.dma_start(t[:], h16_d[c * P:(c + 1) * P, :])
            h_t.append(t)

        def wsc_c(c):
            return wscb[:, c * P:(c + 1) * P]

        def wga_c(c):
            return wgab[:, c * P:(c + 1) * P]

        def scatter_window(w):
            """Segment-sum window w's 20 chunks -> mean [g, d] -> meanT f16."""
            ps = psum.tile([P, D], f32, tag="sc")
            for i in range(CH_PER_WIN):
                c = w * CH_PER_WIN + i
                nc.tensor.matmul(ps[:], lhsT=wsc_c(c), rhs=h_t[c][:],
                                 start=(i == 0), stop=(i == CH_PER_WIN - 1))
            mean = work.tile([P, D], f16, tag="mean")
            nc.scalar.activation(mean[:], ps[:], AF.Copy,
                                 scale=invc_t[:, w:w + 1])
            pst = psum.tile([P, D], f16, tag="tr")
            for b in range(DBLK):
                nc.tensor.transpose(pst[:, b * P:(b + 1) * P],
                                    mean[:, b * P:(b + 1) * P], ident[:])
            meanT = work.tile([P, D], f16, tag="meanT")
            nc.scalar.activation(meanT[:], pst[:], AF.Copy)
            return meanT

        def x2_window(meanT, w, layer):
            """x2 = meanT.T @ Wsum + (bfc+bsum), as f16 [g, d]."""
            ps = psum.tile([P, D], f32, tag="x2")
            for b in range(DBLK):
                nc.tensor.matmul(ps[:], lhsT=meanT[:, b * P:(b + 1) * P],
                                 rhs=wsum_t[layer][b][:],
                                 start=(b == 0), stop=False)
            nc.tensor.matmul(ps[:], lhsT=ones[:, :P], rhs=cbias_t[layer][:],
                             start=False, stop=True)
            x2w = x2pool.tile([P, D], f16, tag=f"x2w{w}", name=f"x2w{w}", bufs=1)
            nc.scalar.activation(x2w[:], ps[:], AF.Copy)
            return x2w

        for layer in range(N_LAYERS):
            # scatter + x2 per window (pipelined one window deep)
            meanTs = {}
            x2ws = {}
            for w in range(WIN_PER_CORE):
                meanTs[w] = scatter_window(w)
                if w >= 1:
                    x2ws[w - 1] = x2_window(meanTs.pop(w - 1), w - 1, layer)
            x2ws[WIN_PER_CORE - 1] = x2_window(
                meanTs.pop(WIN_PER_CORE - 1), WIN_PER_CORE - 1, layer)

            # update pass
            for c in range(CHUNKS):
                w = c // CH_PER_WIN
                pst = psum.tile([P, D], f16, tag="tr")
                for b in range(DBLK):
                    nc.tensor.transpose(pst[:, b * P:(b + 1) * P],
                                        h_t[c][:, b * P:(b + 1) * P], ident[:])
                hTt = stream.tile([P, D], f16, tag="hT", name=f"hT{c}")
                nc.scalar.activation(hTt[:], pst[:], AF.Copy)
                ps = psx1.tile([P, D], f32, tag="x1")
                for b in range(DBLK):
                    nc.tensor.matmul(ps[:], lhsT=hTt[:, b * P:(b + 1) * P],
                                     rhs=wfc_t[layer][b][:],
                                     start=(b == 0), stop=False)
                nc.tensor.matmul(ps[:], lhsT=wga_c(c), rhs=x2ws[w][:],
                                 start=False, stop=True)
                # ELU: h = relu(z) - 1 + exp(min(z, 0))
                t1 = work.tile([P, D], f16, tag="t1")
                nc.scalar.activation(t1[:], ps[:], AF.Relu, scale=-1.0)
                t2 = work.tile([P, D], f16, tag="t2")
                nc.scalar.activation(t2[:], t1[:], AF.Exp, scale=-1.0)
                pm1 = work.tile([P, D], f16, tag="pm1")
                nc.vector.tensor_scalar(pm1[:], ps[:], 0.0, -1.0,
                                        op0=ALU.max, op1=ALU.add)
                nc.vector.tensor_tensor(h_t[c][:], pm1[:], t2[:], op=ALU.add)

        # ---- final scatter + head
        out_sb = const.tile([NUM_TASKS, G_LOC], f32, tag="out")
        for w in range(WIN_PER_CORE):
            hgT = scatter_window(w)            # [d, g] blocks, f16
            # t.T = relu(Wf1.T @ hg.T + bf1)  -> [d2, g] blocks
            tT = work.tile([P, D2], f16, tag="tT", bufs=1)
            for half in range(2):
                ps = psx1.tile([P, D], f32, tag="x1")
                for qi in range(4):
                    q = half * 4 + qi
                    sl = slice(qi * P, (qi + 1) * P)
                    for b in range(DBLK):
                        nc.tensor.matmul(ps[:, sl],
                                         lhsT=wf1_t[b][q][:],
                                         rhs=hgT[:, b * P:(b + 1) * P],
                                         start=(b == 0), stop=False)
                    nc.tensor.matmul(ps[:, sl],
                                     lhsT=bf1_t[:, q * P:(q + 1) * P],
                                     rhs=ones[:, :P], start=False, stop=True)
                nc.scalar.activation(tT[:, half * D:(half + 1) * D],
                                     ps[:], AF.Relu)
            pso = psum.tile([NUM_TASKS, P], f32, tag="x2")
            for q in range(D2BLK):
                nc.tensor.matmul(pso[:], lhsT=wf2_t[q][:],
                                 rhs=tT[:, q * P:(q + 1) * P],
                                 start=(q == 0), stop=False)
            nc.tensor.matmul(pso[:], lhsT=bf2_t[:], rhs=ones[:, :P],
                             start=False, stop=True)
            nc.vector.tensor_copy(out_sb[:, w * P:(w + 1) * P], pso[:])
        nc.sync.dma_start(out_d[:, :], out_sb[:])

    nc.compile()
    return nc


# ------------------------------------------------------------------- kernel

def kernel(**inputs):
    h = np.asarray(inputs["h_subgraph"], dtype=np.float32)
    idx = np.asarray(inputs["subgraph_idx_batch"]).astype(np.int64)
    if not np.all(idx[:-1] <= idx[1:]):        # defensive: index must be sorted
        order = np.argsort(idx, kind="stable")
        h, idx = h[order], idx[order]

    counts = np.bincount(idx, minlength=NUM_GRAPHS)
    starts = np.concatenate([[0], np.cumsum(counts)])
    wins = _pack_windows(counts)
    shared = _prep_shared(
        np.asarray(inputs["Wfc"], np.float32), np.asarray(inputs["bfc"], np.float32),
        np.asarray(inputs["Wsum"], np.float32), np.asarray(inputs["bsum"], np.float32),
        np.asarray(inputs["Wf1"], np.float32), np.asarray(inputs["bf1"], np.float32),
        np.asarray(inputs["Wf2"], np.float32), np.asarray(inputs["bf2"], np.float32),
    )

    in_maps = []
    gmaps = []
    for core in range(N_CORES):
        m, gm = _build_core_inputs(h, idx, counts, starts, wins, core, shared)
        in_maps.append(m)
        gmaps.append(gm)

    _cached["in_maps"] = in_maps
    if "nc" not in _cached:
        _cached["nc"] = _build_program()
    nc = _cached["nc"]

    from concourse import bass_utils
    res = bass_utils.run_bass_kernel_spmd(
        nc, in_maps, core_ids=list(range(N_CORES)))

    out = np.zeros((NUM_GRAPHS, NUM_TASKS), dtype=np.float32)
    for core in range(N_CORES):
        o = res.results[core]["out"]           # [10, 640]
        for g, s in gmaps[core]:
            out[g] = o[:, s]
    return out
